# revision 22
# baseline (speedup 1.0000x reference)
"""Bass/Tile TRN2 kernel for bilinear-score attention (score softmax + context).

reference:
    qW     = query @ W                      [B, Tq, Dk]
    weight = qW @ keys^T + mask[:, None, :] [B, Tq, Tk]
    score  = softmax(weight, axis=-1)
    ctx    = score @ values                 [B, Tq, Dv]
    returns (score, ctx)

Sharding: data-parallel over batch B=16 across 8 NeuronCores (2 batches/core).
Numerics: fp16 hi/lo 3-pass matmuls (hh + hl + lh) for both big contractions
(near-fp32 logits); phase 3 uses one float32r x float32r pass (score
reconstructed fp32 from transposed fp16, values truncated to ~12-bit mantissa
by the PE read path).
"""

import os
import sys

import numpy as np

os.environ.setdefault("JAX_COMPILATION_CACHE_DIR", "/tmp/jax_comp_cache")

for _p in ("/opt/trn_rl_repo",):
    if _p not in sys.path and os.path.isdir(_p):
        sys.path.insert(0, _p)

import concourse.bass as bass  # noqa: E402
import concourse.tile as tile  # noqa: E402
from concourse import bacc, mybir  # noqa: E402
from concourse.bass import ds, ts  # noqa: E402
from concourse.bass_utils import run_bass_kernel_spmd  # noqa: E402

import json as _json

OPTS = {
    "drain_copy": True,      # early PSUM->SBUF drain, softmax reads copy
    "vals_swdge": False,     # values load via gpsimd SWDGE
    "stage_hi": "dve",       # engine for hi cast: act|dve|gpsimd
    "stage_lo": "dve",       # engine for lo subtract: dve|gpsimd
    "stage_ring": "sync",    # ring for staging dma: sync|scalar
    "order": "pipelined",    # pipelined|serial
    "interleave_p3b1": False,
    "score_t": "perqt",      # perqt|batched
    "out_ring": "sync",      # ring for score/ctx/s16 outputs
    "pair_stage": True,      # batch staging loads/writes in pairs
    "s16_ring": "sync",
}
if os.environ.get("K_OPTS"):
    OPTS.update(_json.loads(os.environ["K_OPTS"]))

P = 128
T = 1024
NT = T // P  # 8
NB = 2       # batches per core
NCORES = 8
F32 = mybir.dt.float32
F16 = mybir.dt.float16
F32R = mybir.dt.float32r
AX = mybir.AxisListType
AOP = mybir.AluOpType
AF = mybir.ActivationFunctionType


def _eng(nc, name):
    return {"act": nc.scalar, "dve": nc.vector, "gpsimd": nc.gpsimd}[name]


def _ring(nc, name):
    return {"sync": nc.sync, "scalar": nc.scalar, "gpsimd": nc.gpsimd}[name]


def _hi_lo_to_scratch(nc, stage, src_ap, hi_scr, lo_scr):
    """Load fp32 rows, split into fp16 hi/lo, store to DRAM scratch."""
    ring = _ring(nc, OPTS["stage_ring"])
    hi_e = _eng(nc, OPTS["stage_hi"])
    lo_e = _eng(nc, OPTS["stage_lo"])
    if not OPTS["pair_stage"]:
        for t in range(NT):
            xf = stage.tile([P, T], F32, tag="ldf32", bufs=1)
            ring.dma_start(xf[:], src_ap[ts(t, P), :])
            xh = stage.tile([P, T], F16, tag="hi16")
            if hi_e is nc.scalar:
                nc.scalar.copy(xh[:], xf[:])
            else:
                hi_e.tensor_copy(xh[:], xf[:])
            xl = stage.tile([P, T], F16, tag="lo16")
            lo_e.tensor_tensor(xl[:], xf[:], xh[:], AOP.subtract)
            ring.dma_start(hi_scr[ts(t, P), :], xh[:])
            ring.dma_start(lo_scr[ts(t, P), :], xl[:])
        return
    for t2 in range(NT // 2):
        xf = stage.tile([P, 2, T], F32, tag="ldf32", bufs=1)
        ring.dma_start(xf[:], src_ap[ds(t2 * 2 * P, 2 * P), :].rearrange("(o p) e -> p o e", p=P))
        xh = stage.tile([P, 2, T], F16, tag="hi16")
        xl = stage.tile([P, 2, T], F16, tag="lo16")
        for j in range(2):
            if hi_e is nc.scalar:
                nc.scalar.copy(xh[:, j, :], xf[:, j, :])
            else:
                hi_e.tensor_copy(xh[:, j, :], xf[:, j, :])
            lo_e.tensor_tensor(xl[:, j, :], xf[:, j, :], xh[:, j, :], AOP.subtract)
        ring.dma_start(
            hi_scr[ds(t2 * 2 * P, 2 * P), :].rearrange("(o p) e -> p o e", p=P), xh[:]
        )
        ring.dma_start(
            lo_scr[ds(t2 * 2 * P, 2 * P), :].rearrange("(o p) e -> p o e", p=P), xl[:]
        )


def _phase1(nc, pools, b, w_hi_scr, w_lo_scr, qTh, qTl):
    """qWT[e, q] = W^T @ query^T as fp16 hi/lo, 3-pass per psum tile."""
    wtile, qwt_pool, psA = pools["wtile"], pools["qwt"], pools["psA"]
    qWTh = qwt_pool.tile([P, NT, T], F16, tag="qWTh")
    qWTl = qwt_pool.tile([P, NT, T], F16, tag="qWTl")
    for ec in range(4):  # 256-wide e chunks of W
        wch = wtile.tile([P, NT, 256], F16, tag="wch")
        wcl = wtile.tile([P, NT, 256], F16, tag="wcl")
        ring = _ring(nc, OPTS["stage_ring"])
        ring.dma_start(
            wch[:], w_hi_scr[:, ds(ec * 256, 256)].rearrange("(o p) e -> p o e", p=P)
        )
        ring.dma_start(
            wcl[:], w_lo_scr[:, ds(ec * 256, 256)].rearrange("(o p) e -> p o e", p=P)
        )
        for eh in range(2):
            et = ec * 2 + eh
            ps = psA.tile([P, T], F32, tag="psA")
            for dt_ in range(NT):
                lw_h = wch[:, dt_, ds(eh * P, P)]
                lw_l = wcl[:, dt_, ds(eh * P, P)]
                first = dt_ == 0
                last = dt_ == NT - 1
                for qc in range(2):
                    nc.tensor.matmul(
                        ps[:, ds(qc * 512, 512)], lw_h,
                        qTh[:, dt_, ds(qc * 512, 512)], start=first, stop=False,
                    )
                for qc in range(2):
                    nc.tensor.matmul(
                        ps[:, ds(qc * 512, 512)], lw_h,
                        qTl[:, dt_, ds(qc * 512, 512)], start=False, stop=False,
                    )
                for qc in range(2):
                    nc.tensor.matmul(
                        ps[:, ds(qc * 512, 512)], lw_l,
                        qTh[:, dt_, ds(qc * 512, 512)], start=False, stop=last,
                    )
            nc.scalar.copy(qWTh[:, et, :], ps[:])
            nc.vector.tensor_tensor(qWTl[:, et, :], ps[:], qWTh[:, et, :], AOP.subtract)
    return qWTh, qWTl


def _phase2_softmax(nc, pools, b, s_d, qWTh, qWTl, kTh, kTl, ones, mrep, s16_scr,
                    interleave=None):
    """weight[q, k] = qW @ keys^T + mask; softmax rows; write score + fp16 copy."""
    soft, sc_pool, psB = pools["soft"], pools["sc"], pools["psB"]
    for qt_ in range(NT):
        if interleave is not None and qt_ >= 2:
            interleave(qt_ - 2)
        ps2 = psB.tile([P, T], F32, tag="psB")
        for et in range(NT):
            for li, (lhs, rhs) in enumerate(((qWTh, kTh), (qWTh, kTl), (qWTl, kTh))):
                lw = lhs[:, et, ts(qt_, P)]
                for kc in range(2):
                    nc.tensor.matmul(
                        ps2[:, ds(kc * 512, 512)], lw, rhs[:, et, ds(kc * 512, 512)],
                        start=(et == 0 and li == 0), stop=False,
                    )
        for kc in range(2):
            nc.tensor.matmul(
                ps2[:, ds(kc * 512, 512)], ones[:], mrep[:, ds(kc * 512, 512)],
                start=False, stop=True,
            )
        if OPTS["drain_copy"]:
            wsb = soft.tile([P, T], F32, tag="wsb")
            nc.scalar.copy(wsb[:], ps2[:])
        else:
            wsb = ps2
        negmax = soft.tile([P, 1], F32, tag="negmax")
        nc.vector.tensor_reduce(negmax[:], wsb[:], axis=AX.X, op=AOP.max, negate=True)
        expt = soft.tile([P, T], F32, tag="expt")
        sumexp = soft.tile([P, 1], F32, tag="sumexp")
        nc.scalar.activation(
            expt[:], wsb[:], AF.Exp, bias=negmax[:], scale=1.0, accum_out=sumexp[:]
        )
        recip = soft.tile([P, 1], F32, tag="recip")
        nc.vector.reciprocal(recip[:], sumexp[:])
        nc.vector.tensor_scalar_mul(expt[:], expt[:], recip[:])
        _ring(nc, OPTS["out_ring"]).dma_start(s_d[b, ts(qt_, P), :], expt[:])
        s16 = sc_pool.tile([P, T], F16, tag="s16t")
        nc.scalar.copy(s16[:], expt[:])
        _ring(nc, OPTS["s16_ring"]).dma_start(s16_scr[ts(qt_, P), :], s16[:])
    if interleave is not None:
        for qt_ in range(NT - 2, NT):
            interleave(qt_)


def _phase3_qt(nc, pools, b, c_d, s16_scr, vals, qt_):
    """ctx[qt block] = score @ values via one f32r x f32r pass."""
    st_pool, str_pool, cx_pool, psA = pools["st"], pools["str"], pools["cx"], pools["psA"]
    sT16 = st_pool.tile([P, NT, P], F16, tag="sT16")
    for kt_ in range(NT):
        nc.sync.dma_start_transpose(sT16[:, kt_, :], s16_scr[ts(qt_, P), ts(kt_, P)])
    sTr = str_pool.tile([P, NT, P], F32R, tag="sTr")
    nc.vector.tensor_copy(sTr[:], sT16[:])
    ps3 = psA.tile([P, T], F32, tag="psA")
    for kt_ in range(NT):
        lw = sTr[:, kt_, :]
        for vc in range(2):
            nc.tensor.matmul(
                ps3[:, ds(vc * 512, 512)], lw, vals[:, kt_, ds(vc * 512, 512)],
                start=(kt_ == 0), stop=(kt_ == NT - 1),
            )
    cx = cx_pool.tile([P, T], F32, tag="cx")
    nc.scalar.copy(cx[:], ps3[:])
    _ring(nc, OPTS["out_ring"]).dma_start(c_d[b, ts(qt_, P), :], cx[:])


def _phase3(nc, pools, b, c_d, s16_scr, vals):
    if OPTS["score_t"] == "perqt":
        for qt_ in range(NT):
            _phase3_qt(nc, pools, b, c_d, s16_scr, vals, qt_)
        return
    st_pool, str_pool, cx_pool, psA = pools["st"], pools["str"], pools["cx"], pools["psA"]
    sTall = st_pool.tile([P, NT, T], F16, tag="sTall", bufs=1)
    for kt_ in range(NT):
        nc.sync.dma_start_transpose(sTall[:, kt_, :], s16_scr[:, ts(kt_, P)])
    for qt_ in range(NT):
        sTr = str_pool.tile([P, NT, P], F32R, tag="sTr")
        nc.vector.tensor_copy(sTr[:], sTall[:, :, ts(qt_, P)])
        ps3 = psA.tile([P, T], F32, tag="psA")
        for kt_ in range(NT):
            lw = sTr[:, kt_, :]
            for vc in range(2):
                nc.tensor.matmul(
                    ps3[:, ds(vc * 512, 512)], lw, vals[:, kt_, ds(vc * 512, 512)],
                    start=(kt_ == 0), stop=(kt_ == NT - 1),
                )
        cx = cx_pool.tile([P, T], F32, tag="cx")
        nc.scalar.copy(cx[:], ps3[:])
        _ring(nc, OPTS["out_ring"]).dma_start(c_d[b, ts(qt_, P), :], cx[:])


def _stage_batch(nc, pools, b, tensors):
    q_d, k_d, v_d, m_d, s_d, c_d = tensors
    stage, small, dram = pools["stage"], pools["small"], pools["dram"]
    qt_pool, kt_pool = pools["qt"], pools["kt"]

    # mask -> fp16 broadcast to all partitions
    mf = stage.tile([P, T], F32, tag="ldf32", bufs=1)
    _ring(nc, OPTS["stage_ring"]).dma_start(mf[:1, :], m_d[b : b + 1, :])
    m16 = stage.tile([1, T], F16, tag="lo16")
    nc.vector.tensor_copy(m16[:], mf[:1, :])
    mrep = small.tile([P, T], F16, tag="mrep")
    nc.gpsimd.partition_broadcast(mrep[:], m16[:])

    # query first: stage + transpose, so phase 1 can start ASAP
    q_hi_scr = dram.tile([T, T], F16, tag="qhi")
    q_lo_scr = dram.tile([T, T], F16, tag="qlo")
    _hi_lo_to_scratch(nc, stage, q_d[b], q_hi_scr, q_lo_scr)
    qTh = qt_pool.tile([P, NT, T], F16, tag="qTh")
    qTl = qt_pool.tile([P, NT, T], F16, tag="qTl")
    for dt_ in range(NT):
        nc.sync.dma_start_transpose(qTh[:, dt_, :], q_hi_scr[:, ts(dt_, P)])
        nc.sync.dma_start_transpose(qTl[:, dt_, :], q_lo_scr[:, ts(dt_, P)])

    k_hi_scr = dram.tile([T, T], F16, tag="khi")
    k_lo_scr = dram.tile([T, T], F16, tag="klo")
    _hi_lo_to_scratch(nc, stage, k_d[b], k_hi_scr, k_lo_scr)
    kTh = kt_pool.tile([P, NT, T], F16, tag="kTh")
    kTl = kt_pool.tile([P, NT, T], F16, tag="kTl")
    for dt_ in range(NT):
        nc.sync.dma_start_transpose(kTh[:, dt_, :], k_hi_scr[:, ts(dt_, P)])
        nc.sync.dma_start_transpose(kTl[:, dt_, :], k_lo_scr[:, ts(dt_, P)])
    return qTh, qTl, kTh, kTl, mrep


def _load_values(nc, pools, b, v_d):
    vals = pools["val"].tile([P, NT, T], F32R, tag="vals")
    if OPTS["vals_swdge"]:
        nc.gpsimd.dma_start(vals[:], v_d[b].rearrange("(o p) v -> p o v", p=P))
    else:
        for kt_ in range(NT):
            _ring(nc, OPTS["out_ring"]).dma_start(vals[:, kt_, :], v_d[b, ts(kt_, P), :])
    return vals


PHASE_MARKS = []


def _mark(nc, label):
    PHASE_MARKS.append((int(nc.next_id()), label))


def build_nc(reps=1):
    PHASE_MARKS.clear()
    nc = bacc.Bacc("TRN2", target_bir_lowering=False, debug=False, num_devices=NCORES)
    q_d = nc.dram_tensor("query", [NB, T, T], F32, kind="ExternalInput")
    k_d = nc.dram_tensor("keys", [NB, T, T], F32, kind="ExternalInput")
    v_d = nc.dram_tensor("values", [NB, T, T], F32R, kind="ExternalInput")
    w_d = nc.dram_tensor("W", [T, T], F32, kind="ExternalInput")
    m_d = nc.dram_tensor("mask", [NB, T], F32, kind="ExternalInput")
    s_d = nc.dram_tensor("score", [NB, T, T], F32, kind="ExternalOutput")
    c_d = nc.dram_tensor("ctx", [NB, T, T], F32, kind="ExternalOutput")

    with tile.TileContext(nc) as tc:
        with (
            tc.tile_pool(name="stage", bufs=2) as stage,
            tc.tile_pool(name="wtile", bufs=2) as wtile,
            tc.tile_pool(name="qt", bufs=1) as qt_pool,
            tc.tile_pool(name="qwt", bufs=1) as qwt_pool,
            tc.tile_pool(name="kt", bufs=1) as kt_pool,
            tc.tile_pool(name="val", bufs=1) as val_pool,
            tc.tile_pool(name="soft", bufs=2) as soft,
            tc.tile_pool(name="sc", bufs=2) as sc_pool,
            tc.tile_pool(name="st", bufs=2) as st_pool,
            tc.tile_pool(name="str", bufs=2) as str_pool,
            tc.tile_pool(name="cx", bufs=1) as cx_pool,
            tc.tile_pool(name="small", bufs=1) as small,
            tc.tile_pool(name="ones", bufs=1) as ones_pool,
        ):
            with (
                tc.tile_pool(name="psA", bufs=2, space="PSUM") as psA,
                tc.tile_pool(name="psB", bufs=2, space="PSUM") as psB,
                tc.tile_pool(name="dram", bufs=2, space="DRAM") as dram,
                tc.tile_pool(name="dramw", bufs=1, space="DRAM") as dramw,
            ):
                pools = {
                    "stage": stage, "wtile": wtile, "qt": qt_pool, "qwt": qwt_pool,
                    "kt": kt_pool, "val": val_pool, "soft": soft, "sc": sc_pool,
                    "st": st_pool, "str": str_pool, "cx": cx_pool, "small": small,
                    "psA": psA, "psB": psB, "dram": dram,
                }
                ones = ones_pool.tile([P, P], F16)
                nc.vector.memset(ones[:], 1.0 / P)

                # W -> hi/lo fp16 DRAM scratch (once per core)
                w_hi_scr = dramw.tile([T, T], F16)
                w_lo_scr = dramw.tile([T, T], F16)
                _hi_lo_to_scratch(nc, stage, w_d, w_hi_scr, w_lo_scr)

                tensors = (q_d, k_d, v_d, m_d, s_d, c_d)
                for _rep in range(reps):
                    _mark(nc, "setupW-done")
                    if OPTS["order"] == "pipelined":
                        st0 = _stage_batch(nc, pools, 0, tensors)
                        _mark(nc, "stage0")
                        qWT0 = _phase1(nc, pools, 0, w_hi_scr, w_lo_scr, st0[0], st0[1])
                        _mark(nc, "p1b0")
                        s16_scr0 = dram.tile([T, T], F16, tag="s16")
                        _phase2_softmax(nc, pools, 0, s_d, qWT0[0], qWT0[1],
                                        st0[2], st0[3], ones, st0[4], s16_scr0)
                        _mark(nc, "p2b0")
                        st1 = _stage_batch(nc, pools, 1, tensors)
                        _mark(nc, "stage1")
                        qWT1 = _phase1(nc, pools, 1, w_hi_scr, w_lo_scr, st1[0], st1[1])
                        _mark(nc, "p1b1")
                        vals0 = _load_values(nc, pools, 0, v_d)
                        _phase3(nc, pools, 0, c_d, s16_scr0, vals0)
                        _mark(nc, "p3b0")
                        s16_scr1 = dram.tile([T, T], F16, tag="s16")
                        vals1 = _load_values(nc, pools, 1, v_d)
                        if OPTS["interleave_p3b1"]:
                            _phase2_softmax(
                                nc, pools, 1, s_d, qWT1[0], qWT1[1],
                                st1[2], st1[3], ones, st1[4], s16_scr1,
                                interleave=lambda qt_: _phase3_qt(
                                    nc, pools, 1, c_d, s16_scr1, vals1, qt_
                                ),
                            )
                        else:
                            _phase2_softmax(nc, pools, 1, s_d, qWT1[0], qWT1[1],
                                            st1[2], st1[3], ones, st1[4], s16_scr1)
                            _phase3(nc, pools, 1, c_d, s16_scr1, vals1)
                        _mark(nc, "p2b1+p3b1")
                    else:
                        for b in range(NB):
                            stb = _stage_batch(nc, pools, b, tensors)
                            _mark(nc, f"stage{b}")
                            qWTb = _phase1(nc, pools, b, w_hi_scr, w_lo_scr, stb[0], stb[1])
                            _mark(nc, f"p1b{b}")
                            s16_scrb = dram.tile([T, T], F16, tag="s16")
                            _phase2_softmax(nc, pools, b, s_d, qWTb[0], qWTb[1],
                                            stb[2], stb[3], ones, stb[4], s16_scrb)
                            _mark(nc, f"p2b{b}")
                            valsb = _load_values(nc, pools, b, v_d)
                            _phase3(nc, pools, b, c_d, s16_scrb, valsb)
                            _mark(nc, f"p3b{b}")

    nc.compile()
    return nc


_nc = None


def _get_nc():
    global _nc
    if _nc is None:
        _nc = build_nc()
    return _nc


def make_in_maps(query, keys, values, W, mask):
    query = np.ascontiguousarray(np.asarray(query, dtype=np.float32))
    keys = np.ascontiguousarray(np.asarray(keys, dtype=np.float32))
    values = np.ascontiguousarray(np.asarray(values, dtype=np.float32))
    W = np.ascontiguousarray(np.asarray(W, dtype=np.float32))
    mask = np.ascontiguousarray(np.asarray(mask, dtype=np.float32))
    in_maps = []
    for c in range(NCORES):
        sl = slice(c * NB, (c + 1) * NB)
        in_maps.append(
            {
                "query": query[sl],
                "keys": keys[sl],
                "values": values[sl],
                "W": W,
                "mask": mask[sl],
            }
        )
    return in_maps


def kernel(query, keys, values, W, mask):
    nc = _get_nc()
    in_maps = make_in_maps(query, keys, values, W, mask)
    res = run_bass_kernel_spmd(nc, in_maps, core_ids=list(range(NCORES)))
    score = np.concatenate([res.results[c]["score"] for c in range(NCORES)], axis=0)
    ctx = np.concatenate([res.results[c]["ctx"] for c in range(NCORES)], axis=0)
    return score, ctx


# revision 23
# speedup vs baseline: 31.1106x; 31.1106x over previous
"""Bass/Tile TRN2 kernel for bilinear-score attention (score softmax + context).

reference:
    qW     = query @ W                      [B, Tq, Dk]
    weight = qW @ keys^T + mask[:, None, :] [B, Tq, Tk]
    score  = softmax(weight, axis=-1)
    ctx    = score @ values                 [B, Tq, Dv]
    returns (score, ctx)

Sharding: data-parallel over batch B=16 across 8 NeuronCores (2 batches/core).
Numerics: fp16 hi/lo 3-pass matmuls (hh + hl + lh) for both big contractions
(near-fp32 logits); phase 3 uses one float32r x float32r pass (score
reconstructed fp32 from transposed fp16, values truncated to ~12-bit mantissa
by the PE read path).
"""

import os
import sys

import numpy as np

os.environ.setdefault("JAX_COMPILATION_CACHE_DIR", "/tmp/jax_comp_cache")

for _p in ("/opt/trn_rl_repo",):
    if _p not in sys.path and os.path.isdir(_p):
        sys.path.insert(0, _p)

import concourse.bass as bass  # noqa: E402
import concourse.tile as tile  # noqa: E402
from concourse import bacc, mybir  # noqa: E402
from concourse.bass import ds, ts  # noqa: E402
from concourse.bass_utils import run_bass_kernel_spmd  # noqa: E402

import json as _json

OPTS = {
    "drain_copy": True,      # early PSUM->SBUF drain, softmax reads copy
    "vals_swdge": False,     # values load via gpsimd SWDGE
    "stage_hi": "dve",       # engine for hi cast: act|dve|gpsimd
    "stage_lo": "dve",       # engine for lo subtract: dve|gpsimd
    "stage_ring": "sync",    # ring for staging dma: sync|scalar
    "order": "pipelined",    # pipelined|serial
    "interleave_p3b1": False,
    "score_t": "perqt",      # perqt|batched
    "out_ring": "sync",      # ring for score/ctx/s16 outputs
    "pair_stage": True,      # batch staging loads/writes in pairs
    "s16_ring": "sync",
}
if os.environ.get("K_OPTS"):
    OPTS.update(_json.loads(os.environ["K_OPTS"]))

P = 128
T = 1024
NT = T // P  # 8
NB = 2       # batches per core
NCORES = 8
F32 = mybir.dt.float32
F16 = mybir.dt.float16
F32R = mybir.dt.float32r
AX = mybir.AxisListType
AOP = mybir.AluOpType
AF = mybir.ActivationFunctionType


def _eng(nc, name):
    return {"act": nc.scalar, "dve": nc.vector, "gpsimd": nc.gpsimd}[name]


def _ring(nc, name):
    return {"sync": nc.sync, "scalar": nc.scalar, "gpsimd": nc.gpsimd}[name]


def _hi_lo_to_scratch(nc, stage, src_ap, hi_scr, lo_scr):
    """Load fp32 rows, split into fp16 hi/lo, store to DRAM scratch."""
    ring = _ring(nc, OPTS["stage_ring"])
    hi_e = _eng(nc, OPTS["stage_hi"])
    lo_e = _eng(nc, OPTS["stage_lo"])
    if not OPTS["pair_stage"]:
        for t in range(NT):
            xf = stage.tile([P, T], F32, tag="ldf32", bufs=1)
            ring.dma_start(xf[:], src_ap[ts(t, P), :])
            xh = stage.tile([P, T], F16, tag="hi16")
            if hi_e is nc.scalar:
                nc.scalar.copy(xh[:], xf[:])
            else:
                hi_e.tensor_copy(xh[:], xf[:])
            xl = stage.tile([P, T], F16, tag="lo16")
            lo_e.tensor_tensor(xl[:], xf[:], xh[:], AOP.subtract)
            ring.dma_start(hi_scr[ts(t, P), :], xh[:])
            ring.dma_start(lo_scr[ts(t, P), :], xl[:])
        return
    for t2 in range(NT // 2):
        xf = stage.tile([P, 2, T], F32, tag="ldf32", bufs=1)
        ring.dma_start(xf[:], src_ap[ds(t2 * 2 * P, 2 * P), :].rearrange("(o p) e -> p o e", p=P))
        xh = stage.tile([P, 2, T], F16, tag="hi16")
        xl = stage.tile([P, 2, T], F16, tag="lo16")
        for j in range(2):
            if hi_e is nc.scalar:
                nc.scalar.copy(xh[:, j, :], xf[:, j, :])
            else:
                hi_e.tensor_copy(xh[:, j, :], xf[:, j, :])
            lo_e.tensor_tensor(xl[:, j, :], xf[:, j, :], xh[:, j, :], AOP.subtract)
        ring.dma_start(
            hi_scr[ds(t2 * 2 * P, 2 * P), :].rearrange("(o p) e -> p o e", p=P), xh[:]
        )
        ring.dma_start(
            lo_scr[ds(t2 * 2 * P, 2 * P), :].rearrange("(o p) e -> p o e", p=P), xl[:]
        )


def _phase1(nc, pools, b, w_hi_scr, w_lo_scr, qTh, qTl):
    """qWT[e, q] = W^T @ query^T as fp16 hi/lo, 3-pass per psum tile."""
    wtile, qwt_pool, psA = pools["wtile"], pools["qwt"], pools["psA"]
    qWTh = qwt_pool.tile([P, NT, T], F16, tag="qWTh")
    qWTl = qwt_pool.tile([P, NT, T], F16, tag="qWTl")
    for ec in range(4):  # 256-wide e chunks of W
        wch = wtile.tile([P, NT, 256], F16, tag="wch")
        wcl = wtile.tile([P, NT, 256], F16, tag="wcl")
        ring = _ring(nc, OPTS["stage_ring"])
        ring.dma_start(
            wch[:], w_hi_scr[:, ds(ec * 256, 256)].rearrange("(o p) e -> p o e", p=P)
        )
        ring.dma_start(
            wcl[:], w_lo_scr[:, ds(ec * 256, 256)].rearrange("(o p) e -> p o e", p=P)
        )
        for eh in range(2):
            et = ec * 2 + eh
            ps = psA.tile([P, T], F32, tag="psA")
            for dt_ in range(NT):
                lw_h = wch[:, dt_, ds(eh * P, P)]
                lw_l = wcl[:, dt_, ds(eh * P, P)]
                first = dt_ == 0
                last = dt_ == NT - 1
                for qc in range(2):
                    nc.tensor.matmul(
                        ps[:, ds(qc * 512, 512)], lw_h,
                        qTh[:, dt_, ds(qc * 512, 512)], start=first, stop=False,
                    )
                for qc in range(2):
                    nc.tensor.matmul(
                        ps[:, ds(qc * 512, 512)], lw_h,
                        qTl[:, dt_, ds(qc * 512, 512)], start=False, stop=False,
                    )
                for qc in range(2):
                    nc.tensor.matmul(
                        ps[:, ds(qc * 512, 512)], lw_l,
                        qTh[:, dt_, ds(qc * 512, 512)], start=False, stop=last,
                    )
            nc.scalar.copy(qWTh[:, et, :], ps[:])
            nc.vector.tensor_tensor(qWTl[:, et, :], ps[:], qWTh[:, et, :], AOP.subtract)
    return qWTh, qWTl


def _phase2_softmax(nc, pools, b, s_d, qWTh, qWTl, kTh, kTl, ones, mrep, s16_scr,
                    interleave=None):
    """weight[q, k] = qW @ keys^T + mask; softmax rows; write score + fp16 copy."""
    soft, sc_pool, psB = pools["soft"], pools["sc"], pools["psB"]
    for qt_ in range(NT):
        if interleave is not None and qt_ >= 2:
            interleave(qt_ - 2)
        ps2 = psB.tile([P, T], F32, tag="psB")
        for et in range(NT):
            for li, (lhs, rhs) in enumerate(((qWTh, kTh), (qWTh, kTl), (qWTl, kTh))):
                lw = lhs[:, et, ts(qt_, P)]
                for kc in range(2):
                    nc.tensor.matmul(
                        ps2[:, ds(kc * 512, 512)], lw, rhs[:, et, ds(kc * 512, 512)],
                        start=(et == 0 and li == 0), stop=False,
                    )
        for kc in range(2):
            nc.tensor.matmul(
                ps2[:, ds(kc * 512, 512)], ones[:], mrep[:, ds(kc * 512, 512)],
                start=False, stop=True,
            )
        if OPTS["drain_copy"]:
            wsb = soft.tile([P, T], F32, tag="wsb")
            nc.scalar.copy(wsb[:], ps2[:])
        else:
            wsb = ps2
        negmax = soft.tile([P, 1], F32, tag="negmax")
        nc.vector.tensor_reduce(negmax[:], wsb[:], axis=AX.X, op=AOP.max, negate=True)
        expt = soft.tile([P, T], F32, tag="expt")
        sumexp = soft.tile([P, 1], F32, tag="sumexp")
        nc.scalar.activation(
            expt[:], wsb[:], AF.Exp, bias=negmax[:], scale=1.0, accum_out=sumexp[:]
        )
        recip = soft.tile([P, 1], F32, tag="recip")
        nc.vector.reciprocal(recip[:], sumexp[:])
        nc.vector.tensor_scalar_mul(expt[:], expt[:], recip[:])
        _ring(nc, OPTS["out_ring"]).dma_start(s_d[b, ts(qt_, P), :], expt[:])
        s16 = sc_pool.tile([P, T], F16, tag="s16t")
        nc.scalar.copy(s16[:], expt[:])
        _ring(nc, OPTS["s16_ring"]).dma_start(s16_scr[ts(qt_, P), :], s16[:])
    if interleave is not None:
        for qt_ in range(NT - 2, NT):
            interleave(qt_)


def _phase3_qt(nc, pools, b, c_d, s16_scr, vals, qt_):
    """ctx[qt block] = score @ values via one f32r x f32r pass."""
    st_pool, str_pool, cx_pool, psA = pools["st"], pools["str"], pools["cx"], pools["psA"]
    sT16 = st_pool.tile([P, NT, P], F16, tag="sT16")
    for kt_ in range(NT):
        nc.sync.dma_start_transpose(sT16[:, kt_, :], s16_scr[ts(qt_, P), ts(kt_, P)])
    sTr = str_pool.tile([P, NT, P], F32R, tag="sTr")
    nc.vector.tensor_copy(sTr[:], sT16[:])
    ps3 = psA.tile([P, T], F32, tag="psA")
    for kt_ in range(NT):
        lw = sTr[:, kt_, :]
        for vc in range(2):
            nc.tensor.matmul(
                ps3[:, ds(vc * 512, 512)], lw, vals[:, kt_, ds(vc * 512, 512)],
                start=(kt_ == 0), stop=(kt_ == NT - 1),
            )
    cx = cx_pool.tile([P, T], F32, tag="cx")
    nc.scalar.copy(cx[:], ps3[:])
    _ring(nc, OPTS["out_ring"]).dma_start(c_d[b, ts(qt_, P), :], cx[:])


def _phase3(nc, pools, b, c_d, s16_scr, vals):
    if OPTS["score_t"] == "perqt":
        for qt_ in range(NT):
            _phase3_qt(nc, pools, b, c_d, s16_scr, vals, qt_)
        return
    st_pool, str_pool, cx_pool, psA = pools["st"], pools["str"], pools["cx"], pools["psA"]
    sTall = st_pool.tile([P, NT, T], F16, tag="sTall", bufs=1)
    for kt_ in range(NT):
        nc.sync.dma_start_transpose(sTall[:, kt_, :], s16_scr[:, ts(kt_, P)])
    for qt_ in range(NT):
        sTr = str_pool.tile([P, NT, P], F32R, tag="sTr")
        nc.vector.tensor_copy(sTr[:], sTall[:, :, ts(qt_, P)])
        ps3 = psA.tile([P, T], F32, tag="psA")
        for kt_ in range(NT):
            lw = sTr[:, kt_, :]
            for vc in range(2):
                nc.tensor.matmul(
                    ps3[:, ds(vc * 512, 512)], lw, vals[:, kt_, ds(vc * 512, 512)],
                    start=(kt_ == 0), stop=(kt_ == NT - 1),
                )
        cx = cx_pool.tile([P, T], F32, tag="cx")
        nc.scalar.copy(cx[:], ps3[:])
        _ring(nc, OPTS["out_ring"]).dma_start(c_d[b, ts(qt_, P), :], cx[:])


def _stage_q(nc, pools, b, q_d):
    stage, dram = pools["stage"], pools["dram"]
    qt_pool = pools["qt"]
    q_hi_scr = dram.tile([T, T], F16, tag="qhi")
    q_lo_scr = dram.tile([T, T], F16, tag="qlo")
    _hi_lo_to_scratch(nc, stage, q_d[b], q_hi_scr, q_lo_scr)
    qTh = qt_pool.tile([P, NT, T], F16, tag="qTh")
    qTl = qt_pool.tile([P, NT, T], F16, tag="qTl")
    for dt_ in range(NT):
        nc.sync.dma_start_transpose(qTh[:, dt_, :], q_hi_scr[:, ts(dt_, P)])
        nc.sync.dma_start_transpose(qTl[:, dt_, :], q_lo_scr[:, ts(dt_, P)])
    return qTh, qTl


def _stage_rest(nc, pools, b, tensors):
    q_d, k_d, v_d, m_d, s_d, c_d = tensors
    stage, small, dram = pools["stage"], pools["small"], pools["dram"]
    kt_pool = pools["kt"]

    # mask -> fp16 broadcast to all partitions
    mf = stage.tile([P, T], F32, tag="ldf32", bufs=1)
    _ring(nc, OPTS["stage_ring"]).dma_start(mf[:1, :], m_d[b : b + 1, :])
    m16 = stage.tile([1, T], F16, tag="lo16")
    nc.vector.tensor_copy(m16[:], mf[:1, :])
    mrep = small.tile([P, T], F16, tag="mrep")
    nc.gpsimd.partition_broadcast(mrep[:], m16[:])

    k_hi_scr = dram.tile([T, T], F16, tag="khi")
    k_lo_scr = dram.tile([T, T], F16, tag="klo")
    _hi_lo_to_scratch(nc, stage, k_d[b], k_hi_scr, k_lo_scr)
    kTh = kt_pool.tile([P, NT, T], F16, tag="kTh")
    kTl = kt_pool.tile([P, NT, T], F16, tag="kTl")
    for dt_ in range(NT):
        nc.sync.dma_start_transpose(kTh[:, dt_, :], k_hi_scr[:, ts(dt_, P)])
        nc.sync.dma_start_transpose(kTl[:, dt_, :], k_lo_scr[:, ts(dt_, P)])
    return kTh, kTl, mrep


def _stage_batch(nc, pools, b, tensors):
    qTh, qTl = _stage_q(nc, pools, b, tensors[0])
    kTh, kTl, mrep = _stage_rest(nc, pools, b, tensors)
    return qTh, qTl, kTh, kTl, mrep


def _load_values(nc, pools, b, v_d):
    vals = pools["val"].tile([P, NT, T], F32R, tag="vals")
    if OPTS["vals_swdge"]:
        nc.gpsimd.dma_start(vals[:], v_d[b].rearrange("(o p) v -> p o v", p=P))
    else:
        for kt_ in range(NT):
            _ring(nc, OPTS["out_ring"]).dma_start(vals[:, kt_, :], v_d[b, ts(kt_, P), :])
    return vals


PHASE_MARKS = []


def _mark(nc, label):
    PHASE_MARKS.append((int(nc.next_id()), label))


def build_nc(reps=1):
    PHASE_MARKS.clear()
    nc = bacc.Bacc("TRN2", target_bir_lowering=False, debug=False, num_devices=NCORES)
    q_d = nc.dram_tensor("query", [NB, T, T], F32, kind="ExternalInput")
    k_d = nc.dram_tensor("keys", [NB, T, T], F32, kind="ExternalInput")
    v_d = nc.dram_tensor("values", [NB, T, T], F32R, kind="ExternalInput")
    w_d = nc.dram_tensor("W", [T, T], F32, kind="ExternalInput")
    m_d = nc.dram_tensor("mask", [NB, T], F32, kind="ExternalInput")
    s_d = nc.dram_tensor("score", [NB, T, T], F32, kind="ExternalOutput")
    c_d = nc.dram_tensor("ctx", [NB, T, T], F32, kind="ExternalOutput")

    with tile.TileContext(nc) as tc:
        with (
            tc.tile_pool(name="stage", bufs=2) as stage,
            tc.tile_pool(name="wtile", bufs=2) as wtile,
            tc.tile_pool(name="qt", bufs=1) as qt_pool,
            tc.tile_pool(name="qwt", bufs=1) as qwt_pool,
            tc.tile_pool(name="kt", bufs=1) as kt_pool,
            tc.tile_pool(name="val", bufs=1) as val_pool,
            tc.tile_pool(name="soft", bufs=2) as soft,
            tc.tile_pool(name="sc", bufs=2) as sc_pool,
            tc.tile_pool(name="st", bufs=2) as st_pool,
            tc.tile_pool(name="str", bufs=2) as str_pool,
            tc.tile_pool(name="cx", bufs=1) as cx_pool,
            tc.tile_pool(name="small", bufs=1) as small,
            tc.tile_pool(name="ones", bufs=1) as ones_pool,
        ):
            with (
                tc.tile_pool(name="psA", bufs=2, space="PSUM") as psA,
                tc.tile_pool(name="psB", bufs=2, space="PSUM") as psB,
                tc.tile_pool(name="dram", bufs=2, space="DRAM") as dram,
                tc.tile_pool(name="dramw", bufs=1, space="DRAM") as dramw,
            ):
                pools = {
                    "stage": stage, "wtile": wtile, "qt": qt_pool, "qwt": qwt_pool,
                    "kt": kt_pool, "val": val_pool, "soft": soft, "sc": sc_pool,
                    "st": st_pool, "str": str_pool, "cx": cx_pool, "small": small,
                    "psA": psA, "psB": psB, "dram": dram,
                }
                ones = ones_pool.tile([P, P], F16)
                nc.vector.memset(ones[:], 1.0 / P)

                tensors = (q_d, k_d, v_d, m_d, s_d, c_d)
                qT0 = _stage_q(nc, pools, 0, q_d)

                # W -> hi/lo fp16 DRAM scratch (once per core)
                w_hi_scr = dramw.tile([T, T], F16)
                w_lo_scr = dramw.tile([T, T], F16)
                _hi_lo_to_scratch(nc, stage, w_d, w_hi_scr, w_lo_scr)

                for _rep in range(reps):
                    _mark(nc, "setupW-done")
                    if OPTS["order"] == "pipelined":
                        if _rep == 0:
                            st0 = qT0 + _stage_rest(nc, pools, 0, tensors)
                        else:
                            st0 = _stage_batch(nc, pools, 0, tensors)
                        _mark(nc, "stage0")
                        qWT0 = _phase1(nc, pools, 0, w_hi_scr, w_lo_scr, st0[0], st0[1])
                        _mark(nc, "p1b0")
                        s16_scr0 = dram.tile([T, T], F16, tag="s16")
                        _phase2_softmax(nc, pools, 0, s_d, qWT0[0], qWT0[1],
                                        st0[2], st0[3], ones, st0[4], s16_scr0)
                        _mark(nc, "p2b0")
                        st1 = _stage_batch(nc, pools, 1, tensors)
                        _mark(nc, "stage1")
                        qWT1 = _phase1(nc, pools, 1, w_hi_scr, w_lo_scr, st1[0], st1[1])
                        _mark(nc, "p1b1")
                        vals0 = _load_values(nc, pools, 0, v_d)
                        _phase3(nc, pools, 0, c_d, s16_scr0, vals0)
                        _mark(nc, "p3b0")
                        s16_scr1 = dram.tile([T, T], F16, tag="s16")
                        vals1 = _load_values(nc, pools, 1, v_d)
                        if OPTS["interleave_p3b1"]:
                            _phase2_softmax(
                                nc, pools, 1, s_d, qWT1[0], qWT1[1],
                                st1[2], st1[3], ones, st1[4], s16_scr1,
                                interleave=lambda qt_: _phase3_qt(
                                    nc, pools, 1, c_d, s16_scr1, vals1, qt_
                                ),
                            )
                        else:
                            _phase2_softmax(nc, pools, 1, s_d, qWT1[0], qWT1[1],
                                            st1[2], st1[3], ones, st1[4], s16_scr1)
                            _phase3(nc, pools, 1, c_d, s16_scr1, vals1)
                        _mark(nc, "p2b1+p3b1")
                    else:
                        for b in range(NB):
                            stb = _stage_batch(nc, pools, b, tensors)
                            _mark(nc, f"stage{b}")
                            qWTb = _phase1(nc, pools, b, w_hi_scr, w_lo_scr, stb[0], stb[1])
                            _mark(nc, f"p1b{b}")
                            s16_scrb = dram.tile([T, T], F16, tag="s16")
                            _phase2_softmax(nc, pools, b, s_d, qWTb[0], qWTb[1],
                                            stb[2], stb[3], ones, stb[4], s16_scrb)
                            _mark(nc, f"p2b{b}")
                            valsb = _load_values(nc, pools, b, v_d)
                            _phase3(nc, pools, b, c_d, s16_scrb, valsb)
                            _mark(nc, f"p3b{b}")

    nc.compile()
    return nc


_nc = None


def _get_nc():
    global _nc
    if _nc is None:
        _nc = build_nc()
    return _nc


def make_in_maps(query, keys, values, W, mask):
    query = np.ascontiguousarray(np.asarray(query, dtype=np.float32))
    keys = np.ascontiguousarray(np.asarray(keys, dtype=np.float32))
    values = np.ascontiguousarray(np.asarray(values, dtype=np.float32))
    W = np.ascontiguousarray(np.asarray(W, dtype=np.float32))
    mask = np.ascontiguousarray(np.asarray(mask, dtype=np.float32))
    in_maps = []
    for c in range(NCORES):
        sl = slice(c * NB, (c + 1) * NB)
        in_maps.append(
            {
                "query": query[sl],
                "keys": keys[sl],
                "values": values[sl],
                "W": W,
                "mask": mask[sl],
            }
        )
    return in_maps


def kernel(query, keys, values, W, mask):
    nc = _get_nc()
    in_maps = make_in_maps(query, keys, values, W, mask)
    res = run_bass_kernel_spmd(nc, in_maps, core_ids=list(range(NCORES)))
    score = np.concatenate([res.results[c]["score"] for c in range(NCORES)], axis=0)
    ctx = np.concatenate([res.results[c]["ctx"] for c in range(NCORES)], axis=0)
    return score, ctx
